# revision 2
# baseline (speedup 1.0000x reference)
"""Trainium2 Bass kernel: single-head causal attention.

B=4, T=4096, E=512, H=64, fp32 in/out.

Sharding: 2 cores per batch sample, split by keys (even/odd 128-strips via
a per-256-block half rotation baked in on the host). Each core computes a
partial softmax (numerator + denominator via a ones-column in V) over its
half of the keys for all 4096 queries; the host combines
out = (num0+num1)/(den0+den1).

Device kernel (per core):
  - x streams in per-e-strip (16 x 256KB DMAs) on the sync queue so the
    first kv/q projection matmuls start ~1.7us in; small tensors (wq,
    biases, masks) ride the scalar HWDGE queue in parallel.
  - HAM clock-gate ramp: junk matmuls from t~0.3us until the first x
    strip lands, then bare LDWEIGHTS ops bridge the per-strip DMA gaps,
    keeping the PE activity window busy so the K=4/8 -> 8/8 un-throttle
    fires ~5us in (baseline: 21us).
  - Q projection is single-width (64 rows); the partition-64:128 copy
    that feeds the row-tiled score matmuls is made by a gpsimd SBUF DMA
    (like K's kd tiles), saving ~3.4us of PE time vs the [Wq|Wq] trick.
  - Scores (contraction H=64) run as two concurrent row-tiled matmuls
    (tile_position (0,0)/(64,0) auto-derived from base partitions).
  - PV split into two 64-row matmuls (key halves) accumulating into two
    PSUM banks; halves summed on the host. Whole attention stream stays
    in 64-row PE tiling mode (no mode-switch drains).
  - Output partials evacuated as fp16 with a 2^-6 scale folded in (the
    scale cancels in the host's num/den division).
  - Diagonal trim: 768 of 1024 score columns computed for the diagonal
    pair (exact causal area at strip granularity).
  - exp on the scalar engine with fused 1/sqrt(H) scale (no max
    subtraction; scores are bounded). Scalar ACTIVATE (~38.5us) is the
    pacing engine target.
  - The DIAGONAL pair leads each chunk so the mask multiply (vector
    engine) stays off the chunk-boundary dependency chain.
  - V^T -> V transposes: PE transpose for the first two kv chunks,
    DMA-xbar transpose on sync-queue slack for the rest.
"""

import functools

import numpy as np
import ml_dtypes

B, T, E, H = 4, 4096, 512, 64
NCORES = 8
NCHUNK = 8  # 512-query chunks per sample
CHUNK = T // NCHUNK  # 512
NSTRIP = 16  # local 128-key strips per core (half of T/128)
VSTRIDE = 80  # per-strip stride in the packed V tile
NWARM = 14  # PE warm-up junk matmuls before the first real matmul
NLDW = 4  # bare LDWEIGHTS between early es-strip matmuls (HAM filler)
PACKED_FROM = 2  # chunks >= this use row-tiled scores (dup ready by then)

bf16 = ml_dtypes.bfloat16


@functools.lru_cache(maxsize=1)
def _build():
    import concourse.mybir as mybir
    from concourse import bacc
    import concourse.tile as tile
    from concourse.masks import make_identity

    dt_bf = mybir.dt.bfloat16
    dt_f32 = mybir.dt.float32

    nc = bacc.Bacc("TRN2", target_bir_lowering=False, num_devices=NCORES)

    # x^T, rotated, (quarter, e-strip)-blocked: [4, 128, 4, 1024]
    xt = nc.dram_tensor("xt", [4, 128, 4, T // 4], dt_bf, kind="ExternalInput")
    wq = nc.dram_tensor("wq", [128, 4 * 64], dt_bf, kind="ExternalInput")
    wkv = nc.dram_tensor("wkv", [128, 4 * 128], dt_bf, kind="ExternalInput")
    bias_q = nc.dram_tensor("bias_q", [64, 1], dt_f32, kind="ExternalInput")
    bias_kv = nc.dram_tensor("bias_kv", [128, 1], dt_f32, kind="ExternalInput")
    masks = nc.dram_tensor("masks", [128, 768], dt_bf, kind="ExternalInput")
    # per chunk: [key-half-0 partial | key-half-1 partial], host adds them.
    # fp16 with a 2^-6 scale folded in (cancels in the host's num/den).
    dt_f16 = mybir.dt.float16
    out_d = nc.dram_tensor("out", [H + 1, 2 * T], dt_f16, kind="ExternalOutput")

    scale = 1.0 / float(np.sqrt(H))

    with tile.TileContext(nc) as tc:
        with (
            tc.tile_pool(name="const", bufs=1) as cpool,
            tc.tile_pool(name="xt_pool", bufs=1) as xpool,
            tc.tile_pool(name="q_pool", bufs=3) as qpool,
            tc.tile_pool(name="qd_pool", bufs=2) as qdpool,
            tc.tile_pool(name="kv_pool", bufs=4) as kvpool,
            tc.tile_pool(name="kd_pool", bufs=4) as kdpool,
            tc.tile_pool(name="v_pool", bufs=1) as vpool,
            tc.tile_pool(name="p_pool", bufs=4) as ppool,
            tc.tile_pool(name="o_pool", bufs=2) as opool,
            tc.tile_pool(name="ps_proj", bufs=2, space="PSUM") as pspr,
            tc.tile_pool(name="ps_s", bufs=2, space="PSUM") as pss,
            tc.tile_pool(name="ps_o", bufs=1, space="PSUM") as pso,
        ):
            # ---- input DMAs: x streams per-e-strip on the sync queue in
            # dependency order; small tensors ride the scalar HWDGE queue
            # so they don't delay x ----
            xt_sb = xpool.tile([128, 4 * T], dt_bf)

            def xt_block(qd, es):
                off = (qd * 4 + es) * 1024
                return xt_sb[:, off : off + 1024]

            def xt_dma_es(eng, qd, es):
                eng.dma_start(
                    xt_sb[:, (qd * 4 + es) * 1024 : (qd * 4 + es + 1) * 1024],
                    xt.ap()[qd][:, es, :],
                )

            wkv_sb = cpool.tile([128, 512], dt_bf)
            nc.sync.dma_start(wkv_sb, wkv.ap())
            for es in range(4):
                xt_dma_es(nc.sync, 0, es)
            wq_sb = cpool.tile([128, 256], dt_bf)
            nc.scalar.dma_start(wq_sb, wq.ap())
            bkv_sb = cpool.tile([128, 1], dt_f32)
            nc.scalar.dma_start(bkv_sb, bias_kv.ap())
            bq_sb = cpool.tile([64, 1], dt_f32)
            nc.scalar.dma_start(bq_sb, bias_q.ap())
            masks_sb = cpool.tile([128, 768], dt_bf)
            nc.scalar.dma_start(masks_sb, masks.ap())
            for qd in range(1, 4):
                for es in range(4):
                    xt_dma_es(nc.sync, qd, es)

            # ---- PE warm-up: junk matmuls flip the HAM clock gate to
            # 8/8 while the first x strips land ----
            zt = cpool.tile([128, 128], dt_bf)
            nc.vector.memset(zt, 0.0)
            ident = cpool.tile([128, 128], dt_bf)
            make_identity(nc, ident)
            ps_w = pspr.tile([128, 512], dt_f32, tag="proj")
            for _ in range(NWARM):
                nc.tensor.matmul(ps_w[:, 0:128], lhsT=zt, rhs=zt, start=True, stop=True)

            def pe_filler(n):
                # bare LDWEIGHTS: keeps the PE activity monitor busy
                # across DMA-arrival gaps without touching PSUM
                for _ in range(n):
                    nc.tensor.ldweights(zt[:, :])

            # packed V (natural [k,h] layout + ones column for denominator)
            v_nat = vpool.tile([128, NSTRIP * VSTRIDE], dt_bf)
            v3 = v_nat.rearrange("p (s c) -> p s c", c=VSTRIDE)
            nc.vector.memset(v3[:, :, 64:65], 1.0)

            kv_tiles = []
            kd_tiles = []
            q_tiles = []
            qd_tiles = {}

            def kv_proj(ckv, ldw_fill=False):
                ps_kv = pspr.tile([128, 512], dt_f32, tag="proj")
                for es in range(4):
                    # keys: first 128 tokens of each 256-block
                    key_rhs = xt_block(ckv, es).rearrange(
                        "p (a two b) -> p a two b", two=2, b=128
                    )[:, :, 0, :]
                    nc.tensor.matmul(
                        ps_kv,
                        lhsT=wkv_sb[:, es * 128 : (es + 1) * 128],
                        rhs=key_rhs,
                        start=(es == 0),
                        stop=(es == 3),
                    )
                    if ldw_fill and es < 3:
                        pe_filler(NLDW)
                kv_sb = kvpool.tile([128, 512], dt_bf, tag="kv")
                nc.vector.tensor_scalar_add(kv_sb, ps_kv, bkv_sb)
                kv_tiles.append(kv_sb)
                # K^T duplicate at partitions 64:128 for row-tiled scores
                kd = kdpool.tile([128, 512], dt_bf, tag="kd")
                nc.gpsimd.dma_start(kd[64:128, :], kv_sb[0:64, :])
                kd_tiles.append(kd)
                # V^T blocks -> natural V strips. Early kv chunks go via
                # the PE (needed within ~1us); late ones via the DMA xbar.
                for j in range(4):
                    s = 4 * ckv + j
                    if ckv < 2:
                        ps_tr = pspr.tile([128, 128], dt_bf, tag="proj")
                        nc.tensor.transpose(
                            ps_tr, kv_sb[:, j * 128 : (j + 1) * 128], ident
                        )
                        nc.vector.tensor_copy(
                            v_nat[:, s * VSTRIDE : s * VSTRIDE + 64],
                            ps_tr[:, 64:128],
                        )
                    else:
                        nc.sync.dma_start(
                            v_nat[:, s * VSTRIDE : s * VSTRIDE + 64],
                            kv_sb[64:128, j * 128 : (j + 1) * 128],
                            transpose=True,
                        )

            def q_proj(c, ldw_fill=False):
                ps_q = pspr.tile([64, 512], dt_f32, tag="proj")
                for es in range(4):
                    nc.tensor.matmul(
                        ps_q,
                        lhsT=wq_sb[:, es * 64 : (es + 1) * 64],
                        rhs=xt_block(c // 2, es)[
                            :, (c % 2) * CHUNK : (c % 2) * CHUNK + CHUNK
                        ],
                        start=(es == 0),
                        stop=(es == 3),
                    )
                    if ldw_fill and es < 3:
                        pe_filler(NLDW)
                q_sb = qpool.tile([64, 512], dt_bf, tag="q")
                nc.vector.tensor_scalar_add(q_sb, ps_q, bq_sb)
                q_tiles.append(q_sb)
                if c >= PACKED_FROM:
                    # Q duplicate at partitions 64:128 for row-tiled scores
                    qd = qdpool.tile([128, 512], dt_bf, tag="qd")
                    nc.gpsimd.dma_start(qd[64:128, :], q_sb[0:64, :])
                    qd_tiles[c] = qd

            def emit_S(c, g):
                """Scores for strip pair g of chunk c: strip 2g (512 query
                cols) and strip 2g+1 (256 cols if diagonal, else 512)."""
                diag = g == c
                w2 = 256 if diag else 512
                ps = pss.tile([128, 1024], dt_f32, tag="pss")
                q = q_tiles[c]
                l0, l1 = 2 * g, 2 * g + 1
                lt0 = kv_tiles[l0 // 4][0:64, (l0 % 4) * 128 : (l0 % 4 + 1) * 128]
                if c >= PACKED_FROM:
                    # concurrent row-tiled pair: (0,0) and (64,0)
                    lt1 = kd_tiles[l1 // 4][64:128, (l1 % 4) * 128 : (l1 % 4 + 1) * 128]
                    r1 = qd_tiles[c][64:128, 512 - w2 : 512]
                else:
                    lt1 = kv_tiles[l1 // 4][0:64, (l1 % 4) * 128 : (l1 % 4 + 1) * 128]
                    r1 = q[0:64, 512 - w2 : 512]
                nc.tensor.matmul(
                    ps[:, 0:512], lhsT=lt0, rhs=q[0:64, :], start=True, stop=True
                )
                nc.tensor.matmul(
                    ps[:, 512 : 512 + w2], lhsT=lt1, rhs=r1, start=True, stop=True
                )
                return ps

            def emit_E(c, g, ps):
                diag = g == c
                w = 768 if diag else 1024
                p = ppool.tile([128, 1024], dt_bf, tag="p")
                nc.scalar.activation(
                    p[:, 0:w],
                    ps[:, 0:w],
                    mybir.ActivationFunctionType.Exp,
                    scale=scale,
                )
                if diag:
                    nc.vector.tensor_mul(p[:, 0:768], p[:, 0:768], masks_sb)
                return p

            def emit_V(c, g, p, pso_t, first, last):
                """PV for strip pair g, split into key halves h0/h1 (two
                concurrent 64-row matmuls into separate PSUM banks).
                first/last flag the chunk's accumulation group bounds."""
                diag = g == c
                w2 = 256 if diag else 512
                for i, (l, pc0, pc1, oc0) in enumerate(
                    (
                        (2 * g, 0, 512, 0),
                        (2 * g + 1, 512, 512 + w2, 512 - w2),
                    )
                ):
                    start = first and i == 0
                    stop = last and i == 1
                    vs = v_nat[:, l * VSTRIDE : l * VSTRIDE + 65]
                    nc.tensor.matmul(
                        pso_t[:, oc0:512],
                        lhsT=vs[0:64, :],
                        rhs=p[0:64, pc0:pc1],
                        start=start,
                        stop=stop,
                    )
                    nc.tensor.matmul(
                        pso_t[:, 512 + oc0 : 1024],
                        lhsT=vs[64:128, :],
                        rhs=p[64:128, pc0:pc1],
                        start=start,
                        stop=stop,
                    )

            def emit_O(c, pso_t):
                # single-PSUM-input copy (DVE has one PSUM read port); the
                # host adds the two key-half partials
                o = opool.tile([H + 1, 1024], dt_f16, tag="o")
                nc.vector.tensor_scalar_mul(o, pso_t, 2.0**-6)
                nc.sync.dma_start(out_d.ap()[:, c * 1024 : (c + 1) * 1024], o)

            def proj_filler(c):
                # emitted after the diag PV of chunk c; fills PE idle slots
                # (kv2 moved to c=1 so its DMA-xbar V transposes queue on
                # the sync queue well before chunk 4 needs them)
                if c == 0:
                    kv_proj(1)
                    q_proj(2)
                elif c == 1:
                    q_proj(3)
                    kv_proj(2)
                elif c == 2:
                    q_proj(4)
                elif c == 3:
                    q_proj(5)
                elif c == 4:
                    q_proj(6)
                    kv_proj(3)
                elif c == 5:
                    q_proj(7)

            # ---- software-pipelined main loop: scores run two pairs
            # ahead of exp; PV trails exp by one pair. The DIAGONAL pair
            # goes FIRST in each chunk so the masked PV (the only one
            # gated on the vector engine) is off the chunk-tail chain ----
            all_pairs = [
                (c, g)
                for c in range(NCHUNK)
                for g in ([c] + list(range(c)))
            ]
            ps_map = {}
            sptr = 0

            def pump_S(n):
                nonlocal sptr
                for _ in range(n):
                    if sptr < len(all_pairs):
                        cc, gg = all_pairs[sptr]
                        ps_map[(cc, gg)] = emit_S(cc, gg)
                        sptr += 1

            kv_proj(0, ldw_fill=True)
            q_proj(0, ldw_fill=True)
            q_proj(1, ldw_fill=True)
            pump_S(2)
            pso_t = None
            for i, (c, g) in enumerate(all_pairs):
                first = g == c  # diag pair leads the chunk
                last = (g == c - 1) or (c == 0)
                if first:
                    pso_t = pso.tile([H + 1, 1024], dt_f32, tag="pso")
                p = emit_E(c, g, ps_map.pop((c, g)))
                pump_S(1)
                emit_V(c, g, p, pso_t, first, last)
                if first:
                    proj_filler(c)
                if last:
                    emit_O(c, pso_t)

    nc.compile()
    return nc


def _perm(rho):
    """Rotated-order permutation: rotated position i holds original token
    perm[i]. Involutive (half swap within each 256-block)."""
    i = np.arange(T)
    return (i // 256) * 256 + ((i % 256) + 128 * rho) % 256


def _make_in_maps(x, Wq, bq, Wk, bk, Wv, bv):
    wq_pack = np.ascontiguousarray(
        Wq.reshape(4, 128, 64).transpose(1, 0, 2).reshape(128, 256)
    ).astype(bf16)
    wkv_pack = np.ascontiguousarray(
        np.concatenate([Wk.reshape(4, 128, 64), Wv.reshape(4, 128, 64)], axis=2)
        .transpose(1, 0, 2)
        .reshape(128, 512)
    ).astype(bf16)
    bias_q = np.ascontiguousarray(bq[:, None]).astype(np.float32)
    bias_kv = np.ascontiguousarray(np.concatenate([bk, bv])[:, None]).astype(
        np.float32
    )

    kk = np.arange(128)[:, None]
    in_maps = []
    for b in range(B):
        xt_b = np.ascontiguousarray(x[b].T).astype(bf16).reshape(4, 128, T)
        for rho in range(2):
            perm = _perm(rho)
            xt_rot = xt_b[:, :, perm]  # rotated token order
            xt_in = np.ascontiguousarray(
                xt_rot.reshape(4, 128, 4, T // 4).transpose(2, 1, 0, 3)
            )
            # masks: columns in rotated order; v = original within-chunk
            # offset of rotated column j (chunk-independent). m1 is zero
            # on query cols 0:256 for both cores -> only cols 256:512 kept.
            v = perm[:CHUNK]
            m0 = (kk - v[None, :] <= -128 * rho).astype(bf16)
            m1 = (kk - v[None, :] <= -256 - 128 * rho).astype(bf16)
            masks_np = np.ascontiguousarray(
                np.concatenate([m0, m1[:, 256:512]], axis=1)
            )
            in_maps.append(
                {
                    "xt": xt_in,
                    "wq": wq_pack,
                    "wkv": wkv_pack,
                    "bias_q": bias_q,
                    "bias_kv": bias_kv,
                    "masks": masks_np,
                }
            )
    return in_maps


def _combine(results):
    out = np.empty((B, T, H), np.float32)
    p1 = _perm(1)
    for b in range(B):
        # fold the two key-half partials: [65, 8, 2, 512] -> [65, 4096]
        a0 = (
            results[2 * b]["out"]
            .astype(np.float64)
            .reshape(H + 1, NCHUNK, 2, CHUNK)
            .sum(axis=2)
            .reshape(H + 1, T)
        )
        a1 = (
            results[2 * b + 1]["out"]
            .astype(np.float64)
            .reshape(H + 1, NCHUNK, 2, CHUNK)
            .sum(axis=2)
            .reshape(H + 1, T)
        )
        a1 = a1[:, p1]  # un-rotate core-1 columns (involutive perm)
        num = a0[:H] + a1[:H]
        den = a0[H] + a1[H]
        out[b] = (num / den).T.astype(np.float32)
    return out


def _run(trace=False, **inputs):
    from concourse import bass_utils

    nc = _build()
    in_maps = _make_in_maps(
        np.asarray(inputs["x"], np.float32),
        np.asarray(inputs["Wq"], np.float32),
        np.asarray(inputs["bq"], np.float32),
        np.asarray(inputs["Wk"], np.float32),
        np.asarray(inputs["bk"], np.float32),
        np.asarray(inputs["Wv"], np.float32),
        np.asarray(inputs["bv"], np.float32),
    )
    res = bass_utils.run_bass_kernel_spmd(
        nc, in_maps, list(range(NCORES)), trace=trace
    )
    return _combine(res.results), res.exec_time_ns


def kernel(**inputs):
    out, _ = _run(trace=False, **inputs)
    return out


# revision 3
# speedup vs baseline: 1.0738x; 1.0738x over previous
"""Trainium2 Bass kernel: single-head causal attention.

B=4, T=4096, E=512, H=64, fp32 in/out.

Sharding: 2 cores per batch sample, split by keys (even/odd 128-strips via
a per-256-block half rotation baked in on the host). Each core computes a
partial softmax (numerator + denominator via a ones-column in V) over its
half of the keys for all 4096 queries; the host combines
out = (num0+num1)/(den0+den1).

Device kernel (per core):
  - x streams on the sync queue: quarter 0 per-e-strip (4 DMAs so the
    kv/q projection matmuls pipeline with arrival), quarters 1-3 as
    single 1MB DMAs (DMA-issue instructions cost ~600ns each on the
    sequencer). Small tensors (weights, biases, masks) ride the scalar
    HWDGE queue in parallel.
  - HAM clock-gate ramp: junk matmuls (PSS-pool scratch tiles) run from
    the end of the engine preamble until the first x strip lands, and
    fill the per-strip arrival gaps, so the K=4/8 -> 8/8 un-throttle
    fires ~11us in instead of ~21us. Bare LDWEIGHTS do NOT count as HAM
    activity - only matmuls do.
  - Q projection uses [Wq|Wq] stationary so PSUM rows 0:64 and 64:128
    both hold Q -- the partition-64:128 copy feeds row-tiled scores.
  - K^T duplicates at partitions 64:128 (kd): DVE copy for the first two
    kv chunks (the gpsimd software-DGE queue has multi-us latency and
    would starve chunk-2/3 scores), gpsimd SBUF DMA for the rest.
  - Scores (contraction H=64) run as two concurrent row-tiled matmuls
    (tile_position (0,0)/(64,0) auto-derived from base partitions).
  - PV split into two 64-row matmuls (key halves) accumulating into two
    PSUM banks; halves summed on the host. Whole attention stream stays
    in 64-row PE tiling mode (no mode-switch drains).
  - kv chunk 0's V^T->V PE transposes are DEFERRED until after the first
    score pair: they are not needed until the first PV, and inline they
    add ~2.5us (cold clock) to the x->first-exp critical path.
  - Output partials evacuated as fp16 with a 2^-6 scale folded in (the
    scale cancels in the host's num/den division).
  - Diagonal trim: 768 of 1024 score columns computed for the diag pair.
  - exp on the scalar engine with fused 1/sqrt(H) scale (no max
    subtraction; scores are bounded). Scalar ACTIVATE (~38.5us) is the
    pacing stream once x is resident.
  - The DIAGONAL pair leads each chunk so the mask multiply (vector
    engine) stays off the chunk-boundary dependency chain.
"""

import functools

import numpy as np
import ml_dtypes

B, T, E, H = 4, 4096, 512, 64
NCORES = 8
NCHUNK = 8  # 512-query chunks per sample
CHUNK = T // NCHUNK  # 512
NSTRIP = 16  # local 128-key strips per core (half of T/128)
VSTRIDE = 80  # per-strip stride in the packed V tile
NWARM = 7  # junk matmuls before the first real matmul
NGAP = 2  # junk matmuls per quarter-0 e-strip arrival gap
PACKED_FROM = 2  # chunks >= this use row-tiled scores (kd ready by then)

bf16 = ml_dtypes.bfloat16


@functools.lru_cache(maxsize=1)
def _build():
    import concourse.mybir as mybir
    from concourse import bacc
    import concourse.tile as tile
    from concourse.masks import make_identity

    dt_bf = mybir.dt.bfloat16
    dt_f32 = mybir.dt.float32

    nc = bacc.Bacc("TRN2", target_bir_lowering=False, num_devices=NCORES)

    # x^T, rotated, (quarter, e-strip)-blocked: [4, 128, 4, 1024]
    xt = nc.dram_tensor("xt", [4, 128, 4, T // 4], dt_bf, kind="ExternalInput")
    wq2 = nc.dram_tensor("wq2", [128, 4 * 128], dt_bf, kind="ExternalInput")
    wkv = nc.dram_tensor("wkv", [128, 4 * 128], dt_bf, kind="ExternalInput")
    bias_q2 = nc.dram_tensor("bias_q2", [128, 1], dt_f32, kind="ExternalInput")
    bias_kv = nc.dram_tensor("bias_kv", [128, 1], dt_f32, kind="ExternalInput")
    masks = nc.dram_tensor("masks", [128, 768], dt_bf, kind="ExternalInput")
    # per chunk: [key-half-0 partial | key-half-1 partial], host adds them.
    # fp16 with a 2^-6 scale folded in (cancels in the host's num/den).
    dt_f16 = mybir.dt.float16
    out_d = nc.dram_tensor("out", [H + 1, 2 * T], dt_f16, kind="ExternalOutput")

    scale = 1.0 / float(np.sqrt(H))

    with tile.TileContext(nc) as tc:
        with (
            tc.tile_pool(name="const", bufs=1) as cpool,
            tc.tile_pool(name="xt_pool", bufs=1) as xpool,
            tc.tile_pool(name="q_pool", bufs=3) as qpool,
            tc.tile_pool(name="kv_pool", bufs=4) as kvpool,
            tc.tile_pool(name="kd_pool", bufs=4) as kdpool,
            tc.tile_pool(name="v_pool", bufs=1) as vpool,
            tc.tile_pool(name="p_pool", bufs=4) as ppool,
            tc.tile_pool(name="o_pool", bufs=2) as opool,
            tc.tile_pool(name="ps_proj", bufs=2, space="PSUM") as pspr,
            tc.tile_pool(name="ps_s", bufs=2, space="PSUM") as pss,
            tc.tile_pool(name="ps_o", bufs=1, space="PSUM") as pso,
        ):
            # ---- input DMAs ----
            xt_sb = xpool.tile([128, 4 * T], dt_bf)

            def xt_block(qd, es):
                off = (qd * 4 + es) * 1024
                return xt_sb[:, off : off + 1024]

            wkv_sb = cpool.tile([128, 512], dt_bf)
            nc.sync.dma_start(wkv_sb, wkv.ap())
            # quarter 0 per-e-strip so projection matmuls pipeline with
            # arrival; quarters 1-3 whole (fewer issue slots)
            for es in range(4):
                nc.sync.dma_start(
                    xt_sb[:, es * 1024 : (es + 1) * 1024], xt.ap()[0][:, es, :]
                )
            for qd in range(1, 4):
                nc.sync.dma_start(
                    xt_sb[:, qd * 4096 : (qd + 1) * 4096],
                    xt.ap()[qd].rearrange("p a t -> p (a t)"),
                )
            wq2_sb = cpool.tile([128, 512], dt_bf)
            nc.scalar.dma_start(wq2_sb, wq2.ap())
            bkv_sb = cpool.tile([128, 1], dt_f32)
            nc.scalar.dma_start(bkv_sb, bias_kv.ap())
            bq2_sb = cpool.tile([128, 1], dt_f32)
            nc.scalar.dma_start(bq2_sb, bias_q2.ap())
            masks_sb = cpool.tile([128, 768], dt_bf)
            nc.scalar.dma_start(masks_sb, masks.ap())

            # ---- HAM warm-up: junk matmuls on rotating PSS-pool scratch
            # tiles (safe to emit anywhere - no readers) ----
            zt = cpool.tile([128, 512], dt_bf)
            nc.gpsimd.memset(zt, 0.0)
            ident = cpool.tile([128, 128], dt_bf)
            make_identity(nc, ident)

            def junk(n):
                jt = pss.tile([128, 512], dt_f32, tag="pss")
                for _ in range(n):
                    nc.tensor.matmul(
                        jt, lhsT=zt[:, 0:128], rhs=zt, start=True, stop=True
                    )

            junk(NWARM)

            # packed V (natural [k,h] layout + ones column for denominator)
            v_nat = vpool.tile([128, NSTRIP * VSTRIDE], dt_bf)
            v3 = v_nat.rearrange("p (s c) -> p s c", c=VSTRIDE)
            nc.vector.memset(v3[:, :, 64:65], 1.0)

            kv_tiles = []
            kd_tiles = []
            q_tiles = []

            def v_transpose(ckv, j):
                # V^T block -> natural V strip via PE transpose
                s = 4 * ckv + j
                kv_sb = kv_tiles[ckv]
                ps_tr = pspr.tile([128, 128], dt_bf, tag="proj")
                nc.tensor.transpose(ps_tr, kv_sb[:, j * 128 : (j + 1) * 128], ident)
                nc.vector.tensor_copy(
                    v_nat[:, s * VSTRIDE : s * VSTRIDE + 64],
                    ps_tr[:, 64:128],
                )

            def kv_proj(ckv, defer_tr=False, gap_junk=False):
                ps_kv = pspr.tile([128, 512], dt_f32, tag="proj")
                for es in range(4):
                    # keys: first 128 tokens of each 256-block
                    key_rhs = xt_block(ckv, es).rearrange(
                        "p (a two b) -> p a two b", two=2, b=128
                    )[:, :, 0, :]
                    nc.tensor.matmul(
                        ps_kv,
                        lhsT=wkv_sb[:, es * 128 : (es + 1) * 128],
                        rhs=key_rhs,
                        start=(es == 0),
                        stop=(es == 3),
                    )
                    if gap_junk and es < 3:
                        junk(NGAP)
                kv_sb = kvpool.tile([128, 512], dt_bf, tag="kv")
                nc.vector.tensor_scalar_add(kv_sb, ps_kv, bkv_sb)
                kv_tiles.append(kv_sb)
                # K^T duplicate at partitions 64:128 for row-tiled scores.
                # DVE copy early (gpsimd software-DGE has multi-us latency),
                # gpsimd DMA late (DVE is loaded mid-kernel).
                kd = kdpool.tile([128, 512], dt_bf, tag="kd")
                if ckv < 2:
                    nc.vector.tensor_copy(kd[64:128, :], kv_sb[0:64, :])
                else:
                    nc.gpsimd.dma_start(kd[64:128, :], kv_sb[0:64, :])
                kd_tiles.append(kd)
                # V^T -> V strips: PE transposes for early kv chunks
                # (deferred for chunk 0), DMA xbar for late ones.
                if ckv < 2:
                    if not defer_tr:
                        for j in range(4):
                            v_transpose(ckv, j)
                else:
                    for j in range(4):
                        s = 4 * ckv + j
                        nc.sync.dma_start(
                            v_nat[:, s * VSTRIDE : s * VSTRIDE + 64],
                            kv_sb[64:128, j * 128 : (j + 1) * 128],
                            transpose=True,
                        )

            def q_proj(c):
                ps_q = pspr.tile([128, 512], dt_f32, tag="proj")
                for es in range(4):
                    nc.tensor.matmul(
                        ps_q,
                        lhsT=wq2_sb[:, es * 128 : (es + 1) * 128],
                        rhs=xt_block(c // 2, es)[
                            :, (c % 2) * CHUNK : (c % 2) * CHUNK + CHUNK
                        ],
                        start=(es == 0),
                        stop=(es == 3),
                    )
                q_sb = qpool.tile([128, 512], dt_bf, tag="q")
                nc.vector.tensor_scalar_add(q_sb, ps_q, bq2_sb)
                q_tiles.append(q_sb)

            def emit_S(c, g):
                """Scores for strip pair g of chunk c: strip 2g (512 query
                cols) and strip 2g+1 (256 cols if diagonal, else 512)."""
                diag = g == c
                w2 = 256 if diag else 512
                ps = pss.tile([128, 1024], dt_f32, tag="pss")
                q = q_tiles[c]
                l0, l1 = 2 * g, 2 * g + 1
                lt0 = kv_tiles[l0 // 4][0:64, (l0 % 4) * 128 : (l0 % 4 + 1) * 128]
                if c >= PACKED_FROM:
                    # concurrent row-tiled pair: (0,0) and (64,0)
                    lt1 = kd_tiles[l1 // 4][64:128, (l1 % 4) * 128 : (l1 % 4 + 1) * 128]
                    r1 = q[64:128, 512 - w2 : 512]
                else:
                    lt1 = kv_tiles[l1 // 4][0:64, (l1 % 4) * 128 : (l1 % 4 + 1) * 128]
                    r1 = q[0:64, 512 - w2 : 512]
                nc.tensor.matmul(
                    ps[:, 0:512], lhsT=lt0, rhs=q[0:64, :], start=True, stop=True
                )
                nc.tensor.matmul(
                    ps[:, 512 : 512 + w2], lhsT=lt1, rhs=r1, start=True, stop=True
                )
                return ps

            def emit_E(c, g, ps):
                diag = g == c
                w = 768 if diag else 1024
                p = ppool.tile([128, 1024], dt_bf, tag="p")
                nc.scalar.activation(
                    p[:, 0:w],
                    ps[:, 0:w],
                    mybir.ActivationFunctionType.Exp,
                    scale=scale,
                )
                if diag:
                    nc.vector.tensor_mul(p[:, 0:768], p[:, 0:768], masks_sb)
                return p

            def emit_V(c, g, p, pso_t, first, last):
                """PV for strip pair g, split into key halves h0/h1 (two
                concurrent 64-row matmuls into separate PSUM banks)."""
                diag = g == c
                w2 = 256 if diag else 512
                for i, (l, pc0, pc1, oc0) in enumerate(
                    (
                        (2 * g, 0, 512, 0),
                        (2 * g + 1, 512, 512 + w2, 512 - w2),
                    )
                ):
                    start = first and i == 0
                    stop = last and i == 1
                    vs = v_nat[:, l * VSTRIDE : l * VSTRIDE + 65]
                    nc.tensor.matmul(
                        pso_t[:, oc0:512],
                        lhsT=vs[0:64, :],
                        rhs=p[0:64, pc0:pc1],
                        start=start,
                        stop=stop,
                    )
                    nc.tensor.matmul(
                        pso_t[:, 512 + oc0 : 1024],
                        lhsT=vs[64:128, :],
                        rhs=p[64:128, pc0:pc1],
                        start=start,
                        stop=stop,
                    )

            def emit_O(c, pso_t):
                # single-PSUM-input copy (DVE has one PSUM read port); the
                # host adds the two key-half partials
                o = opool.tile([H + 1, 1024], dt_f16, tag="o")
                nc.vector.tensor_scalar_mul(o, pso_t, 2.0**-6)
                nc.sync.dma_start(out_d.ap()[:, c * 1024 : (c + 1) * 1024], o)

            def proj_filler(c):
                # emitted after the diag PV of chunk c; fills PE idle slots
                if c == 0:
                    kv_proj(1)
                    q_proj(2)
                elif c == 1:
                    q_proj(3)
                    kv_proj(2)
                elif c == 2:
                    q_proj(4)
                elif c == 3:
                    q_proj(5)
                elif c == 4:
                    q_proj(6)
                    kv_proj(3)
                elif c == 5:
                    q_proj(7)

            # ---- software-pipelined main loop: scores run two pairs
            # ahead of exp; PV trails exp by one pair. The DIAGONAL pair
            # goes FIRST in each chunk so the masked PV (the only one
            # gated on the vector engine) is off the chunk-tail chain ----
            all_pairs = [
                (c, g)
                for c in range(NCHUNK)
                for g in ([c] + list(range(c)))
            ]
            ps_map = {}
            sptr = 0

            def pump_S(n):
                nonlocal sptr
                for _ in range(n):
                    if sptr < len(all_pairs):
                        cc, gg = all_pairs[sptr]
                        ps_map[(cc, gg)] = emit_S(cc, gg)
                        sptr += 1

            kv_proj(0, defer_tr=True, gap_junk=True)
            q_proj(0)
            q_proj(1)
            pump_S(2)
            # deferred kv-chunk-0 V transposes: needed by the first PVs,
            # but off the x -> first-exp critical path
            for j in range(4):
                v_transpose(0, j)
            pso_t = None
            for i, (c, g) in enumerate(all_pairs):
                first = g == c  # diag pair leads the chunk
                last = (g == c - 1) or (c == 0)
                if first:
                    pso_t = pso.tile([H + 1, 1024], dt_f32, tag="pso")
                p = emit_E(c, g, ps_map.pop((c, g)))
                pump_S(1)
                emit_V(c, g, p, pso_t, first, last)
                if first:
                    proj_filler(c)
                if last:
                    emit_O(c, pso_t)

    nc.compile()
    return nc


def _perm(rho):
    """Rotated-order permutation: rotated position i holds original token
    perm[i]. Involutive (half swap within each 256-block)."""
    i = np.arange(T)
    return (i // 256) * 256 + ((i % 256) + 128 * rho) % 256


def _make_in_maps(x, Wq, bq, Wk, bk, Wv, bv):
    # [Wq|Wq] per e-strip: the partition-64:128 copy of Q feeds the
    # row-tiled score matmuls.
    wq4 = Wq.reshape(4, 128, 64)
    wq2_pack = np.ascontiguousarray(
        np.concatenate([wq4, wq4], axis=2).transpose(1, 0, 2).reshape(128, 512)
    ).astype(bf16)
    wkv_pack = np.ascontiguousarray(
        np.concatenate([Wk.reshape(4, 128, 64), Wv.reshape(4, 128, 64)], axis=2)
        .transpose(1, 0, 2)
        .reshape(128, 512)
    ).astype(bf16)
    bias_q2 = np.ascontiguousarray(
        np.concatenate([bq, bq])[:, None]
    ).astype(np.float32)
    bias_kv = np.ascontiguousarray(np.concatenate([bk, bv])[:, None]).astype(
        np.float32
    )

    kk = np.arange(128)[:, None]
    in_maps = []
    for b in range(B):
        xt_b = np.ascontiguousarray(x[b].T).astype(bf16).reshape(4, 128, T)
        for rho in range(2):
            perm = _perm(rho)
            xt_rot = xt_b[:, :, perm]  # rotated token order
            xt_in = np.ascontiguousarray(
                xt_rot.reshape(4, 128, 4, T // 4).transpose(2, 1, 0, 3)
            )
            # masks: columns in rotated order; v = original within-chunk
            # offset of rotated column j (chunk-independent). m1 is zero
            # on query cols 0:256 for both cores -> only cols 256:512 kept.
            v = perm[:CHUNK]
            m0 = (kk - v[None, :] <= -128 * rho).astype(bf16)
            m1 = (kk - v[None, :] <= -256 - 128 * rho).astype(bf16)
            masks_np = np.ascontiguousarray(
                np.concatenate([m0, m1[:, 256:512]], axis=1)
            )
            in_maps.append(
                {
                    "xt": xt_in,
                    "wq2": wq2_pack,
                    "wkv": wkv_pack,
                    "bias_q2": bias_q2,
                    "bias_kv": bias_kv,
                    "masks": masks_np,
                }
            )
    return in_maps


def _combine(results):
    out = np.empty((B, T, H), np.float32)
    p1 = _perm(1)
    for b in range(B):
        # fold the two key-half partials: [65, 8, 2, 512] -> [65, 4096]
        a0 = (
            results[2 * b]["out"]
            .astype(np.float64)
            .reshape(H + 1, NCHUNK, 2, CHUNK)
            .sum(axis=2)
            .reshape(H + 1, T)
        )
        a1 = (
            results[2 * b + 1]["out"]
            .astype(np.float64)
            .reshape(H + 1, NCHUNK, 2, CHUNK)
            .sum(axis=2)
            .reshape(H + 1, T)
        )
        a1 = a1[:, p1]  # un-rotate core-1 columns (involutive perm)
        num = a0[:H] + a1[:H]
        den = a0[H] + a1[H]
        out[b] = (num / den).T.astype(np.float32)
    return out


def _run(trace=False, **inputs):
    from concourse import bass_utils

    nc = _build()
    in_maps = _make_in_maps(
        np.asarray(inputs["x"], np.float32),
        np.asarray(inputs["Wq"], np.float32),
        np.asarray(inputs["bq"], np.float32),
        np.asarray(inputs["Wk"], np.float32),
        np.asarray(inputs["bk"], np.float32),
        np.asarray(inputs["Wv"], np.float32),
        np.asarray(inputs["bv"], np.float32),
    )
    res = bass_utils.run_bass_kernel_spmd(
        nc, in_maps, list(range(NCORES)), trace=trace
    )
    return _combine(res.results), res.exec_time_ns


def kernel(**inputs):
    out, _ = _run(trace=False, **inputs)
    return out


# revision 8
# speedup vs baseline: 1.1032x; 1.0273x over previous
"""Trainium2 Bass kernel: single-head causal attention.

B=4, T=4096, E=512, H=64, fp32 in/out.

Sharding: 2 cores per batch sample, split by keys (even/odd 128-strips via
a per-256-block half rotation baked in on the host). Each core computes a
partial softmax (numerator + denominator via a ones-column in V) over its
half of the keys for all 4096 queries; the host combines
out = (num0+num1)/(den0+den1).

Device kernel (per core):
  - x streams on the sync queue: quarter 0 per-e-strip (4 DMAs so the
    kv/q projection matmuls pipeline with arrival), quarters 1-3 as
    single 1MB DMAs (DMA-issue instructions cost ~600ns each on the
    sequencer). Small tensors (weights, biases, masks) ride the scalar
    HWDGE queue in parallel.
  - HAM clock-gate ramp: junk matmuls (PSS-pool scratch tiles) run from
    the end of the engine preamble until the first x strip lands, and
    fill the per-strip arrival gaps, so the K=4/8 -> 8/8 un-throttle
    fires ~11us in instead of ~21us. Bare LDWEIGHTS do NOT count as HAM
    activity - only matmuls do.
  - Q projection uses [Wq|Wq] stationary so PSUM rows 0:64 and 64:128
    both hold Q -- the partition-64:128 copy feeds row-tiled scores.
  - K^T duplicates at partitions 64:128 (kd): DVE copy for the first two
    kv chunks (the gpsimd software-DGE queue has multi-us latency and
    would starve chunk-2/3 scores), gpsimd SBUF DMA for the rest.
  - Scores (contraction H=64) run as two concurrent row-tiled matmuls
    (tile_position (0,0)/(64,0) auto-derived from base partitions).
  - PV split into two 64-row matmuls (key halves) accumulating into two
    PSUM banks; halves summed on the host. Whole attention stream stays
    in 64-row PE tiling mode (no mode-switch drains).
  - kv chunk 0's V^T->V PE transposes are DEFERRED until after the first
    score pair: they are not needed until the first PV, and inline they
    add ~2.5us (cold clock) to the x->first-exp critical path.
  - Output partials evacuated as fp16 with a 2^-6 scale folded in (the
    scale cancels in the host's num/den division).
  - Diagonal trim: 768 of 1024 score columns computed for the diag pair.
  - exp on the scalar engine with fused 1/sqrt(H) scale (no max
    subtraction; scores are bounded). Scalar ACTIVATE (~38.5us) is the
    pacing stream once x is resident.
  - The DIAGONAL pair leads each chunk so the mask multiply (vector
    engine) stays off the chunk-boundary dependency chain.
"""

import functools

import numpy as np
import ml_dtypes

B, T, E, H = 4, 4096, 512, 64
NCORES = 8
NCHUNK = 8  # 512-query chunks per sample
CHUNK = T // NCHUNK  # 512
NSTRIP = 16  # local 128-key strips per core (half of T/128)
VSTRIDE = 80  # per-strip stride in the packed V tile
NWARM = 7  # junk matmuls before the first real matmul
NGAP = 2  # junk matmuls per quarter-0 e-strip arrival gap
PACKED_FROM = 2  # chunks >= this use row-tiled scores (kd ready by then)

bf16 = ml_dtypes.bfloat16


@functools.lru_cache(maxsize=1)
def _build():
    import concourse.mybir as mybir
    from concourse import bacc
    import concourse.tile as tile
    from concourse.masks import make_identity

    dt_bf = mybir.dt.bfloat16
    dt_f32 = mybir.dt.float32

    nc = bacc.Bacc("TRN2", target_bir_lowering=False, num_devices=NCORES)

    # x^T, rotated, (quarter, e-strip)-blocked: [4, 128, 4, 1024]
    xt = nc.dram_tensor("xt", [4, 128, 4, T // 4], dt_bf, kind="ExternalInput")
    wq2 = nc.dram_tensor("wq2", [128, 4 * 128], dt_bf, kind="ExternalInput")
    wkv = nc.dram_tensor("wkv", [128, 4 * 128], dt_bf, kind="ExternalInput")
    bias_q2 = nc.dram_tensor("bias_q2", [128, 1], dt_f32, kind="ExternalInput")
    bias_kv = nc.dram_tensor("bias_kv", [128, 1], dt_f32, kind="ExternalInput")
    masks = nc.dram_tensor("masks", [128, 768], dt_bf, kind="ExternalInput")
    # per chunk: [key-half-0 partial | key-half-1 partial], host adds them.
    # fp16 with a 2^-6 scale folded in (cancels in the host's num/den).
    dt_f16 = mybir.dt.float16
    out_d = nc.dram_tensor("out", [H + 1, 2 * T], dt_f16, kind="ExternalOutput")

    scale = 1.0 / float(np.sqrt(H))

    with tile.TileContext(nc) as tc:
        with (
            tc.tile_pool(name="const", bufs=1) as cpool,
            tc.tile_pool(name="xt_pool", bufs=1) as xpool,
            tc.tile_pool(name="q_pool", bufs=3) as qpool,
            tc.tile_pool(name="kv_pool", bufs=4) as kvpool,
            tc.tile_pool(name="kd_pool", bufs=4) as kdpool,
            tc.tile_pool(name="v_pool", bufs=1) as vpool,
            tc.tile_pool(name="p_pool", bufs=4) as ppool,
            tc.tile_pool(name="o_pool", bufs=2) as opool,
            tc.tile_pool(name="ps_proj", bufs=2, space="PSUM") as pspr,
            tc.tile_pool(name="ps_s", bufs=2, space="PSUM") as pss,
            tc.tile_pool(name="ps_o", bufs=1, space="PSUM") as pso,
        ):
            # ---- input DMAs ----
            xt_sb = xpool.tile([128, 4 * T], dt_bf)

            def xt_block(qd, es):
                off = (qd * 4 + es) * 1024
                return xt_sb[:, off : off + 1024]

            wkv_sb = cpool.tile([128, 512], dt_bf)
            nc.sync.dma_start(wkv_sb, wkv.ap())
            # quarter 0 per-(e-strip, 512-token half) so the chunk-0
            # projection chain starts ~2us after wkv; quarters 1-3 whole
            # (DMA-issue instructions cost ~600ns each on the sequencer)
            for h in range(2):
                for es in range(4):
                    nc.sync.dma_start(
                        xt_sb[:, es * 1024 + h * 512 : es * 1024 + h * 512 + 512],
                        xt.ap()[0][:, es, h * 512 : (h + 1) * 512],
                    )
            for qd in range(1, 4):
                nc.sync.dma_start(
                    xt_sb[:, qd * 4096 : (qd + 1) * 4096],
                    xt.ap()[qd].rearrange("p a t -> p (a t)"),
                )
            bkv_sb = cpool.tile([128, 1], dt_f32)
            nc.scalar.dma_start(bkv_sb, bias_kv.ap())
            bq2_sb = cpool.tile([128, 1], dt_f32)
            nc.scalar.dma_start(bq2_sb, bias_q2.ap())
            wq2_sb = cpool.tile([128, 512], dt_bf)
            nc.scalar.dma_start(wq2_sb, wq2.ap())
            masks_sb = cpool.tile([128, 768], dt_bf)
            nc.scalar.dma_start(masks_sb, masks.ap())

            # ---- HAM warm-up: junk matmuls on rotating PSS-pool scratch
            # tiles (safe to emit anywhere - no readers) ----
            zt = cpool.tile([128, 512], dt_bf)
            nc.gpsimd.memset(zt, 0.0)
            ident = cpool.tile([128, 128], dt_bf)
            make_identity(nc, ident)

            def junk(n):
                jt = pss.tile([128, 512], dt_f32, tag="pss")
                for _ in range(n):
                    nc.tensor.matmul(
                        jt, lhsT=zt[:, 0:128], rhs=zt, start=True, stop=True
                    )

            junk(NWARM)

            # packed V (natural [k,h] layout + ones column for denominator)
            v_nat = vpool.tile([128, NSTRIP * VSTRIDE], dt_bf)
            v3 = v_nat.rearrange("p (s c) -> p s c", c=VSTRIDE)
            nc.vector.memset(v3[:, :, 64:65], 1.0)

            kv_tiles = []
            kd_tiles = []
            q_tiles = []

            def v_transpose(ckv, j):
                # V^T block -> natural V strip via PE transpose
                s = 4 * ckv + j
                kv_sb = kv_tiles[ckv]
                ps_tr = pspr.tile([128, 128], dt_bf, tag="proj")
                nc.tensor.transpose(ps_tr, kv_sb[:, j * 128 : (j + 1) * 128], ident)
                nc.vector.tensor_copy(
                    v_nat[:, s * VSTRIDE : s * VSTRIDE + 64],
                    ps_tr[:, 64:128],
                )

            def kv_finish(ckv, kv_sb):
                # K^T duplicate at partitions 64:128 for row-tiled scores.
                # DVE copy (the gpsimd software-DGE moves ~8GB/s and would
                # starve the row-tiled scores of late chunks).
                kd = kdpool.tile([128, 512], dt_bf, tag="kd")
                nc.vector.tensor_copy(kd[64:128, :], kv_sb[0:64, :])
                kd_tiles.append(kd)
                # V^T -> V strips: PE transposes for early kv chunks
                # (deferred for chunk 0), DMA xbar for late ones.
                if ckv >= 2:
                    for j in range(4):
                        s = 4 * ckv + j
                        nc.sync.dma_start(
                            v_nat[:, s * VSTRIDE : s * VSTRIDE + 64],
                            kv_sb[64:128, j * 128 : (j + 1) * 128],
                            transpose=True,
                        )

            def kv_proj(ckv):
                ps_kv = pspr.tile([128, 512], dt_f32, tag="proj")
                for es in range(4):
                    # keys: first 128 tokens of each 256-block
                    key_rhs = xt_block(ckv, es).rearrange(
                        "p (a two b) -> p a two b", two=2, b=128
                    )[:, :, 0, :]
                    nc.tensor.matmul(
                        ps_kv,
                        lhsT=wkv_sb[:, es * 128 : (es + 1) * 128],
                        rhs=key_rhs,
                        start=(es == 0),
                        stop=(es == 3),
                    )
                kv_sb = kvpool.tile([128, 512], dt_bf, tag="kv")
                nc.vector.tensor_scalar_add(kv_sb, ps_kv, bkv_sb)
                kv_tiles.append(kv_sb)
                if ckv == 1:
                    for j in range(4):
                        v_transpose(1, j)
                kv_finish(ckv, kv_sb)

            def kv_proj0_half(kv0_sb, h):
                # strips 2h, 2h+1 from quarter-0 tokens [h*512:(h+1)*512]
                ps_kvh = pspr.tile([128, 256], dt_f32, tag="proj")
                for es in range(4):
                    blk = xt_block(0, es)[:, h * 512 : (h + 1) * 512]
                    key_rhs = blk.rearrange(
                        "p (a two b) -> p a two b", two=2, b=128
                    )[:, :, 0, :]
                    nc.tensor.matmul(
                        ps_kvh,
                        lhsT=wkv_sb[:, es * 128 : (es + 1) * 128],
                        rhs=key_rhs,
                        start=(es == 0),
                        stop=(es == 3),
                    )
                    if h == 0 and es < 3:
                        junk(NGAP)
                nc.vector.tensor_scalar_add(
                    kv0_sb[:, h * 256 : (h + 1) * 256], ps_kvh, bkv_sb
                )

            def q_proj(c):
                ps_q = pspr.tile([128, 512], dt_f32, tag="proj")
                for es in range(4):
                    nc.tensor.matmul(
                        ps_q,
                        lhsT=wq2_sb[:, es * 128 : (es + 1) * 128],
                        rhs=xt_block(c // 2, es)[
                            :, (c % 2) * CHUNK : (c % 2) * CHUNK + CHUNK
                        ],
                        start=(es == 0),
                        stop=(es == 3),
                    )
                q_sb = qpool.tile([128, 512], dt_bf, tag="q")
                nc.vector.tensor_scalar_add(q_sb, ps_q, bq2_sb)
                q_tiles.append(q_sb)

            def emit_S(c, g):
                """Scores for strip pair g of chunk c: strip 2g (512 query
                cols) and strip 2g+1 (256 cols if diagonal, else 512)."""
                diag = g == c
                w2 = 256 if diag else 512
                ps = pss.tile([128, 1024], dt_f32, tag="pss")
                q = q_tiles[c]
                l0, l1 = 2 * g, 2 * g + 1
                lt0 = kv_tiles[l0 // 4][0:64, (l0 % 4) * 128 : (l0 % 4 + 1) * 128]
                if c >= PACKED_FROM:
                    # concurrent row-tiled pair: (0,0) and (64,0)
                    lt1 = kd_tiles[l1 // 4][64:128, (l1 % 4) * 128 : (l1 % 4 + 1) * 128]
                    r1 = q[64:128, 512 - w2 : 512]
                else:
                    lt1 = kv_tiles[l1 // 4][0:64, (l1 % 4) * 128 : (l1 % 4 + 1) * 128]
                    r1 = q[0:64, 512 - w2 : 512]
                nc.tensor.matmul(
                    ps[:, 0:512], lhsT=lt0, rhs=q[0:64, :], start=True, stop=True
                )
                nc.tensor.matmul(
                    ps[:, 512 : 512 + w2], lhsT=lt1, rhs=r1, start=True, stop=True
                )
                return ps

            def emit_E(c, g, ps):
                diag = g == c
                w = 768 if diag else 1024
                p = ppool.tile([128, 1024], dt_bf, tag="p")
                nc.scalar.activation(
                    p[:, 0:w],
                    ps[:, 0:w],
                    mybir.ActivationFunctionType.Exp,
                    scale=scale,
                )
                if diag:
                    nc.vector.tensor_mul(p[:, 0:768], p[:, 0:768], masks_sb)
                return p

            def emit_V(c, g, p, pso_t, first, last):
                """PV for strip pair g, split into key halves h0/h1 (two
                concurrent 64-row matmuls into separate PSUM banks)."""
                diag = g == c
                w2 = 256 if diag else 512
                for i, (l, pc0, pc1, oc0) in enumerate(
                    (
                        (2 * g, 0, 512, 0),
                        (2 * g + 1, 512, 512 + w2, 512 - w2),
                    )
                ):
                    start = first and i == 0
                    stop = last and i == 1
                    vs = v_nat[:, l * VSTRIDE : l * VSTRIDE + 65]
                    nc.tensor.matmul(
                        pso_t[:, oc0:512],
                        lhsT=vs[0:64, :],
                        rhs=p[0:64, pc0:pc1],
                        start=start,
                        stop=stop,
                    )
                    nc.tensor.matmul(
                        pso_t[:, 512 + oc0 : 1024],
                        lhsT=vs[64:128, :],
                        rhs=p[64:128, pc0:pc1],
                        start=start,
                        stop=stop,
                    )

            def emit_O(c, pso_t):
                # single-PSUM-input copy (DVE has one PSUM read port); the
                # host adds the two key-half partials
                o = opool.tile([H + 1, 1024], dt_f16, tag="o")
                nc.vector.tensor_scalar_mul(o, pso_t, 2.0**-6)
                nc.sync.dma_start(out_d.ap()[:, c * 1024 : (c + 1) * 1024], o)

            def proj_filler(c):
                # emitted at the END of chunk c (a proj matmul whose x
                # quarter is still in flight must never sit in the PE
                # stream ahead of the current chunk's score/PV matmuls)
                if c == 0:
                    kv_proj(1)
                    q_proj(2)
                elif c == 1:
                    q_proj(3)
                    kv_proj(2)
                elif c == 2:
                    q_proj(4)
                elif c == 3:
                    q_proj(5)
                elif c == 4:
                    q_proj(6)
                    kv_proj(3)
                elif c == 5:
                    q_proj(7)

            # ---- software-pipelined main loop: scores run two pairs
            # ahead of exp; PV trails exp by one pair. The DIAGONAL pair
            # goes FIRST in each chunk so the masked PV (the only one
            # gated on the vector engine) is off the chunk-tail chain ----
            all_pairs = [
                (c, g)
                for c in range(NCHUNK)
                for g in ([c] + list(range(c)))
            ]
            ps_map = {}
            sptr = 0

            def pump_S(n):
                nonlocal sptr
                for _ in range(n):
                    if sptr < len(all_pairs):
                        cc, gg = all_pairs[sptr]
                        ps_map[(cc, gg)] = emit_S(cc, gg)
                        sptr += 1

            # chunk-0 half pipeline: strips 0-1 + Q(0) need only the first
            # 512 tokens of quarter 0 -> first score pair ~2us earlier
            kv0_sb = kvpool.tile([128, 512], dt_bf, tag="kv")
            kv_tiles.append(kv0_sb)
            kv_proj0_half(kv0_sb, 0)
            q_proj(0)
            kv_proj0_half(kv0_sb, 1)
            q_proj(1)
            kv_finish(0, kv0_sb)
            pump_S(2)
            # deferred kv-chunk-0 V transposes: needed by the first PVs,
            # but off the x -> first-exp critical path
            for j in range(4):
                v_transpose(0, j)
            pso_t = None
            for i, (c, g) in enumerate(all_pairs):
                first = g == c  # diag pair leads the chunk
                last = (g == c - 1) or (c == 0)
                if first:
                    pso_t = pso.tile([H + 1, 1024], dt_f32, tag="pso")
                p = emit_E(c, g, ps_map.pop((c, g)))
                pump_S(1)
                emit_V(c, g, p, pso_t, first, last)
                if last:
                    emit_O(c, pso_t)
                    proj_filler(c)

    nc.compile()
    return nc


def _perm(rho):
    """Rotated-order permutation: rotated position i holds original token
    perm[i]. Involutive (half swap within each 256-block)."""
    i = np.arange(T)
    return (i // 256) * 256 + ((i % 256) + 128 * rho) % 256


def _make_in_maps(x, Wq, bq, Wk, bk, Wv, bv):
    # [Wq|Wq] per e-strip: the partition-64:128 copy of Q feeds the
    # row-tiled score matmuls.
    wq4 = Wq.reshape(4, 128, 64)
    wq2_pack = np.ascontiguousarray(
        np.concatenate([wq4, wq4], axis=2).transpose(1, 0, 2).reshape(128, 512)
    ).astype(bf16)
    wkv_pack = np.ascontiguousarray(
        np.concatenate([Wk.reshape(4, 128, 64), Wv.reshape(4, 128, 64)], axis=2)
        .transpose(1, 0, 2)
        .reshape(128, 512)
    ).astype(bf16)
    bias_q2 = np.ascontiguousarray(
        np.concatenate([bq, bq])[:, None]
    ).astype(np.float32)
    bias_kv = np.ascontiguousarray(np.concatenate([bk, bv])[:, None]).astype(
        np.float32
    )

    kk = np.arange(128)[:, None]
    in_maps = []
    for b in range(B):
        xt_b = np.ascontiguousarray(x[b].T).astype(bf16).reshape(4, 128, T)
        for rho in range(2):
            perm = _perm(rho)
            xt_rot = xt_b[:, :, perm]  # rotated token order
            xt_in = np.ascontiguousarray(
                xt_rot.reshape(4, 128, 4, T // 4).transpose(2, 1, 0, 3)
            )
            # masks: columns in rotated order; v = original within-chunk
            # offset of rotated column j (chunk-independent). m1 is zero
            # on query cols 0:256 for both cores -> only cols 256:512 kept.
            v = perm[:CHUNK]
            m0 = (kk - v[None, :] <= -128 * rho).astype(bf16)
            m1 = (kk - v[None, :] <= -256 - 128 * rho).astype(bf16)
            masks_np = np.ascontiguousarray(
                np.concatenate([m0, m1[:, 256:512]], axis=1)
            )
            in_maps.append(
                {
                    "xt": xt_in,
                    "wq2": wq2_pack,
                    "wkv": wkv_pack,
                    "bias_q2": bias_q2,
                    "bias_kv": bias_kv,
                    "masks": masks_np,
                }
            )
    return in_maps


def _combine(results):
    out = np.empty((B, T, H), np.float32)
    p1 = _perm(1)
    for b in range(B):
        # fold the two key-half partials: [65, 8, 2, 512] -> [65, 4096]
        a0 = (
            results[2 * b]["out"]
            .astype(np.float64)
            .reshape(H + 1, NCHUNK, 2, CHUNK)
            .sum(axis=2)
            .reshape(H + 1, T)
        )
        a1 = (
            results[2 * b + 1]["out"]
            .astype(np.float64)
            .reshape(H + 1, NCHUNK, 2, CHUNK)
            .sum(axis=2)
            .reshape(H + 1, T)
        )
        a1 = a1[:, p1]  # un-rotate core-1 columns (involutive perm)
        num = a0[:H] + a1[:H]
        den = a0[H] + a1[H]
        out[b] = (num / den).T.astype(np.float32)
    return out


def _run(trace=False, **inputs):
    from concourse import bass_utils

    nc = _build()
    in_maps = _make_in_maps(
        np.asarray(inputs["x"], np.float32),
        np.asarray(inputs["Wq"], np.float32),
        np.asarray(inputs["bq"], np.float32),
        np.asarray(inputs["Wk"], np.float32),
        np.asarray(inputs["bk"], np.float32),
        np.asarray(inputs["Wv"], np.float32),
        np.asarray(inputs["bv"], np.float32),
    )
    res = bass_utils.run_bass_kernel_spmd(
        nc, in_maps, list(range(NCORES)), trace=trace
    )
    return _combine(res.results), res.exec_time_ns


def kernel(**inputs):
    out, _ = _run(trace=False, **inputs)
    return out
